# revision 34
# baseline (speedup 1.0000x reference)
"""NetVLAD-style pooling kernel for Trainium2 (8 NeuronCores, data parallel).

Reference computation (per batch sample b, x: [N=128, D=512]):
    logits = x @ clusters            [N, 48]
    logits = BN(logits)              (inference batchnorm, folded into C'/bias')
    a = softmax(logits)[:, :32]      [N, 32]
    vlad[d, k] = sum_n a[n,k] x[n,d] - (sum_n a[n,k]) * clusters2[d,k]
    vlad /= ||vlad||_2 (over d, per k); flatten; vlad /= ||vlad||_2

Strategy:
  - Pure data parallel: batch 4096 -> 512 samples per core across 8 cores.
  - Host pre-casts x to fp8 e3m4 (x*2 to center the e3m4 range; the scale
    cancels in the per-cluster L2 norm / is folded into cp) in BOTH layouts
    ([n,d] and [d,n] transposed) so the device never transposes; HBM
    traffic/core = 2*32MiB in + 16MiB out (vs 144MiB for the fp16 version).
  - x operands fp8, small operands (cp, bias, a) fp16; psum accumulation
    fp32. BN folded into weights; bias added via a Kc=2 rank-2 matmul
    (hi+lo fp16 split, fp32-accurate).
  - Softmax without max-subtraction (max |logit| ~ 21, exp is safe in fp32).
  - vlad matmul col-tiled 4 samples per PSUM bank; asum via extra -ones
    column; epilogue fused on DVE; final L2 norm is exactly sqrt(32) after
    the per-cluster normalization, so it folds into one scale constant.
  - Per-row norms: Square stays on ACT (square is in every ACT table);
    Sqrt is batched over 8 subgroups (one table swap pair per batch);
    final scale via ACT Copy (table-free) with per-partition scale AP.
  - Output written fp16 in [4s*32k (partition), subgroup, d] layout; the
    host reorders/upcasts (cheap numpy, outside the device kernel).
"""

import numpy as np

B, N, D, KT, K = 4096, 128, 512, 48, 32
NCORES = 8
BLOC = B // NCORES            # 512 samples per core
G = 16                        # samples per DMA group
NT = BLOC // G                # 32 groups
SUBS = G // 4                 # subgroups per group
NB = 8                        # groups per norm batch (32 subgroups)
BN_EPS = 1e-5
XSCALE = 1.0                  # e3m4 covers N(0,1) x directly; folded anyway

_F16 = np.float16


def _f8():
    import ml_dtypes
    return ml_dtypes.float8_e3m4


_NC_CACHE = {}


def _build_nc():
    """Build + compile the Bass/Tile kernel (one NeuronCore's program)."""
    from contextlib import ExitStack

    import concourse.bass as bass
    import concourse.mybir as mybir
    import concourse.tile as tile
    from concourse import bacc

    bf = mybir.dt.float16
    f8 = mybir.dt.float8e3
    f32 = mybir.dt.float32
    AF = mybir.ActivationFunctionType
    ALU = mybir.AluOpType

    nc = bacc.Bacc("TRN2", target_bir_lowering=False, debug=False)

    xp = nc.dram_tensor("xp", [NT, 128, G, D + 512], f8, kind="ExternalInput")
    cp = nc.dram_tensor("cp", [128, 4, KT], bf, kind="ExternalInput")
    bias2 = nc.dram_tensor("bias2", [2, 4 * KT], bf, kind="ExternalInput")
    c2t4 = nc.dram_tensor("c2t4", [128, D], bf, kind="ExternalInput")
    out = nc.dram_tensor("out", [128, SUBS * NT, D], bf, kind="ExternalOutput")

    with tile.TileContext(nc) as tc, ExitStack() as ctx:
        singles = ctx.enter_context(tc.tile_pool(name="singles", bufs=1))
        xp_pool = ctx.enter_context(tc.tile_pool(name="xp", bufs=4))
        exp_pool = ctx.enter_context(tc.tile_pool(name="exp", bufs=4))
        sm_pool = ctx.enter_context(tc.tile_pool(name="small", bufs=6))
        a_pool = ctx.enter_context(tc.tile_pool(name="a", bufs=4))
        # vlc tiles stay alive for a whole 32-subgroup norm batch
        vlc_pool = ctx.enter_context(tc.tile_pool(name="vlc", bufs=36))
        sq_pool = ctx.enter_context(tc.tile_pool(name="sq", bufs=2))
        st_pool = ctx.enter_context(tc.tile_pool(name="st", bufs=2))
        out_pool = ctx.enter_context(tc.tile_pool(name="vout", bufs=2))
        lg_ps = ctx.enter_context(tc.tile_pool(name="lg", bufs=3, space="PSUM"))
        vl_ps = ctx.enter_context(tc.tile_pool(name="vl", bufs=3, space="PSUM"))
        as_ps = ctx.enter_context(tc.tile_pool(name="as", bufs=2, space="PSUM"))

        # params go on the scalar queue so they overlap the first x load
        cp_t = singles.tile([128, 4, KT], bf)
        nc.scalar.dma_start(out=cp_t, in_=cp[:, :, :])
        bias2_t = singles.tile([2, 4 * KT], bf)
        nc.scalar.dma_start(out=bias2_t, in_=bias2[:, :])
        c2_t = singles.tile([128, D], bf)
        nc.scalar.dma_start(out=c2_t, in_=c2t4[:, :])
        ones2_t = singles.tile([2, 128], bf)
        nc.vector.memset(ones2_t, 1.0)
        nones_t = singles.tile([128, 1], bf)
        nc.vector.memset(nones_t, -1.0)
        eps_t = singles.tile([128, 1], f32)
        nc.vector.memset(eps_t, 1e-20)

        # uniform norm batches: one Sqrt table swap pair per 32 subgroups
        sched = [8, 8, 8, 8]
        assert sum(sched) == NT
        t0 = 0
        for nb in sched:
            vlcs = []
            ss_t = st_pool.tile([128, SUBS * NB], f32, tag="ss")
            for t in range(t0, t0 + nb):
                xp_t = xp_pool.tile([128, G, D + 512], f8)
                nc.sync.dma_start(out=xp_t, in_=xp[t, :, :, :])

                for h in range(SUBS):  # 4-sample subgroups per group
                    j = SUBS * (t - t0) + h
                    lg = lg_ps.tile([128, 4, KT], f32)
                    # bias: logits += 1*bias_hi + 1*bias_lo (rank-2 matmul)
                    nc.tensor.matmul(
                        lg[:, :, :], ones2_t[:, :], bias2_t[:, :],
                        start=True, stop=False,
                    )
                    for s in range(4):
                        g = 4 * h + s
                        for c in range(4):
                            nc.tensor.matmul(
                                lg[:, s, :],
                                xp_t[:, g, D + 128 * c : D + 128 * (c + 1)],
                                cp_t[:, c, :],
                                start=False, stop=(c == 3),
                            )
                    # softmax over 48 (free dim), no max subtraction needed
                    exp_t = exp_pool.tile([128, 4, KT], f32)
                    nc.scalar.activation(exp_t, lg[:, :, :], AF.Exp)
                    den_t = sm_pool.tile([128, 4], f32)
                    nc.vector.reduce_sum(den_t, exp_t, axis=mybir.AxisListType.X)
                    rec_t = sm_pool.tile([128, 4], f32)
                    nc.vector.reciprocal(rec_t, den_t)
                    # a = exp[:, :, :K] * recip (one op, free-dim broadcast)
                    a_t = a_pool.tile([128, 4, K], bf)
                    rec_ap = rec_t[:, :]
                    rec_bc = bass.AP(
                        rec_ap.tensor, rec_ap.offset,
                        [rec_ap.ap[0], rec_ap.ap[1], [0, K]],
                    )
                    nc.gpsimd.tensor_mul(a_t[:, :, :], exp_t[:, :, 0:K], rec_bc)
                    # vlad = a^T @ x (col-tiled, 4 samples share one PSUM bank)
                    vl = vl_ps.tile([128, D], f32)
                    asm = as_ps.tile([128, 1], f32)
                    for s in range(4):
                        g = 4 * h + s
                        nc.tensor.matmul(
                            vl[32 * s : 32 * s + 32, :],
                            a_t[:, s, :],
                            xp_t[:, g, 0:D],
                            start=True, stop=True,
                            tile_position=(0, 32 * s),
                        )
                    # -asum for all 4 samples in one matmul (lhsT free = 128)
                    nc.tensor.matmul(
                        asm[:, :], a_t[:, :, :], nones_t[:, :],
                        start=True, stop=True,
                    )
                    # vlc = vl + (-asum)*c2t  (asm holds -asum)
                    vlc_t = vlc_pool.tile([128, D], bf)
                    nc.vector.scalar_tensor_tensor(
                        out=vlc_t, in0=c2_t, scalar=asm[:, 0:1], in1=vl[:, :],
                        op0=ALU.mult, op1=ALU.add,
                    )
                    vlcs.append(vlc_t)
                    # ss[:, j] = sum_d vlc^2 (Square is in every ACT table)
                    sq_t = sq_pool.tile([128, D], bf)
                    nc.scalar.activation(
                        sq_t, vlc_t, AF.Square, accum_out=ss_t[:, j : j + 1]
                    )
            # batched norm: sfac = 1/sqrt(32*ss + tiny); one table swap pair
            nj = SUBS * nb
            den_b = st_pool.tile([128, SUBS * NB], f32, tag="denb")
            nc.scalar.activation(
                den_b[:, 0:nj], ss_t[:, 0:nj], AF.Sqrt, scale=32.0,
                bias=eps_t[:, 0:1],
            )
            sf_b = st_pool.tile([128, SUBS * NB], f32, tag="sfb")
            nc.vector.reciprocal(sf_b[:, 0:nj], den_b[:, 0:nj])
            vn_t = out_pool.tile([128, NB * SUBS, D], bf, tag="vn")
            for j in range(nj):
                nc.vector.tensor_scalar_mul(
                    vn_t[:, j, :], vlcs[j], sf_b[:, j : j + 1]
                )
                # stream the store out in 8-subgroup slices (incl. remainder)
                if j % 8 == 7 or j == nj - 1:
                    j0 = (j // 8) * 8
                    nc.scalar.dma_start(
                        out=out[:, SUBS * t0 + j0 : SUBS * t0 + j + 1, :],
                        in_=vn_t[:, j0 : j + 1, :],
                    )
            t0 += nb

    nc.compile()
    return nc


def _get_nc():
    if "nc" not in _NC_CACHE:
        _NC_CACHE["nc"] = _build_nc()
    return _NC_CACHE["nc"]


def _prep_host(x, clusters, bn_w, bn_b, bn_rm, bn_rv, clusters2):
    """Fold BN into weights; build device-layout arrays."""
    s = (1.0 / np.sqrt(bn_rv.astype(np.float64) + BN_EPS)) * bn_w
    s = s.astype(np.float32)
    Cp = (clusters * s[None, :]).astype(np.float32)
    biasp = (bn_b - bn_rm * s).astype(np.float32)

    cp_dev = np.ascontiguousarray(
        (Cp / XSCALE).reshape(4, 128, KT).transpose(1, 0, 2).astype(_F16)
    )
    b_hi = biasp.astype(_F16)
    b_lo = (biasp - b_hi.astype(np.float32)).astype(_F16)
    bias2_dev = np.ascontiguousarray(
        np.stack([np.tile(b_hi, 4), np.tile(b_lo, 4)], axis=0)
    )
    c2t4_dev = np.ascontiguousarray(
        np.tile((clusters2[0].T * XSCALE).astype(_F16), (4, 1))
    )

    xb = (x * XSCALE).astype(_f8())  # [B, N, D]
    xp_list = []
    for c in range(NCORES):
        xc = xb[c * BLOC : (c + 1) * BLOC]
        xpk = np.empty((NT, 128, G, D + 512), dtype=_f8())
        # natural layout block: partition = n
        xpk[:, :, :, 0:D] = xc.reshape(NT, G, 128, D).transpose(0, 2, 1, 3)
        # transposed layout block: partition = d within 128-chunk
        xpk[:, :, :, D:] = (
            xc.reshape(NT, G, 128, 4, 128)
            .transpose(0, 4, 1, 3, 2)
            .reshape(NT, 128, G, 512)
        )
        xp_list.append(xpk)
    return cp_dev, bias2_dev, c2t4_dev, xp_list


def kernel(x, clusters, bn_w, bn_b, bn_rm, bn_rv, clusters2):
    from concourse.bass_utils import run_bass_kernel_spmd

    cp_dev, bias2_dev, c2t4_dev, xp_list = _prep_host(
        x, clusters, bn_w, bn_b, bn_rm, bn_rv, clusters2
    )
    nc = _get_nc()
    in_maps = [
        {
            "xp": xp_list[c],
            "cp": cp_dev,
            "bias2": bias2_dev,
            "c2t4": c2t4_dev,
        }
        for c in range(NCORES)
    ]
    res = run_bass_kernel_spmd(nc, in_maps, core_ids=list(range(NCORES)))
    outs = []
    for c in range(NCORES):
        o = res.results[c]["out"].astype(np.float32)  # [128, SUBS*NT, D]
        o = o.reshape(4, K, SUBS * NT, D).transpose(2, 0, 3, 1).reshape(BLOC, D * K)
        outs.append(o)
    return np.ascontiguousarray(np.concatenate(outs, axis=0))



# revision 39
# speedup vs baseline: 1.0324x; 1.0324x over previous
"""NetVLAD-style pooling kernel for Trainium2 (8 NeuronCores, data parallel).

Reference computation (per batch sample b, x: [N=128, D=512]):
    logits = x @ clusters            [N, 48]
    logits = BN(logits)              (inference batchnorm, folded into C'/bias')
    a = softmax(logits)[:, :32]      [N, 32]
    vlad[d, k] = sum_n a[n,k] x[n,d] - (sum_n a[n,k]) * clusters2[d,k]
    vlad /= ||vlad||_2 (over d, per k); flatten; vlad /= ||vlad||_2

Strategy:
  - Pure data parallel: batch 4096 -> 512 samples per core across 8 cores.
  - Host pre-casts x to fp8 e3m4 in BOTH layouts ([n,d] natural for the vlad
    matmul, [d,n] transposed for the cluster projection) so the device never
    transposes; HBM traffic/core = 2*32MiB in + 16MiB out (the fp16 dual
    layout was 144MiB; the kernel is HBM-bound, so fp8 x is the big win).
    e3m4 (not e4m3): 4 mantissa bits suit N(0,1) data, ~2x lower error.
  - x operands fp8, small operands (cp, bias, a) fp16 (mixed-dtype matmuls);
    psum accumulation fp32. BN folded into weights; bias added via a Kc=2
    rank-2 matmul (hi+lo fp16 split, fp32-accurate).
  - Softmax without max-subtraction (max |logit| ~ 21, exp is safe in fp32);
    a = exp*recip runs on the otherwise-idle GpSimd engine.
  - vlad matmul col-tiled 4 samples per PSUM bank; -asum for all 4 samples
    in one 128-wide matmul; vlc = vl + (-asum)*c2 fused on DVE (fp16 out);
    final L2 norm is exactly sqrt(32) after the per-cluster normalization,
    so it folds into the per-cluster scale.
  - Per-row norms: Square+accum on ACT; Sqrt batched over 32 subgroups (one
    table swap pair per batch, eps folded in as activation bias); final
    per-partition scale on DVE (fp16 in/out hits the 2x DVE mode); stores
    stream out in 8-subgroup slices.
  - Output written fp16 in [4s*32k (partition), subgroup, d] layout; the
    host reorders/upcasts (cheap numpy, outside the device kernel).
"""

import numpy as np

B, N, D, KT, K = 4096, 128, 512, 48, 32
NCORES = 8
BLOC = B // NCORES            # 512 samples per core
G = 16                        # samples per DMA group
NT = BLOC // G                # 32 groups
SUBS = G // 4                 # subgroups per group
NB = 8                        # groups per norm batch (32 subgroups)
BN_EPS = 1e-5
XSCALE = 1.0                  # e3m4 covers N(0,1) x directly; folded anyway

_F16 = np.float16


def _f8():
    import ml_dtypes
    return ml_dtypes.float8_e3m4


_NC_CACHE = {}


def _build_nc():
    """Build + compile the Bass/Tile kernel (one NeuronCore's program)."""
    from contextlib import ExitStack

    import concourse.bass as bass
    import concourse.mybir as mybir
    import concourse.tile as tile
    from concourse import bacc

    bf = mybir.dt.float16
    f8 = mybir.dt.float8e3
    f32 = mybir.dt.float32
    AF = mybir.ActivationFunctionType
    ALU = mybir.AluOpType

    nc = bacc.Bacc("TRN2", target_bir_lowering=False, debug=False)

    xp = nc.dram_tensor("xp", [NT, 128, G, D + 512], f8, kind="ExternalInput")
    cp = nc.dram_tensor("cp", [128, 4, KT], bf, kind="ExternalInput")
    bias2 = nc.dram_tensor("bias2", [2, 4 * KT], bf, kind="ExternalInput")
    c2t4 = nc.dram_tensor("c2t4", [128, D], bf, kind="ExternalInput")
    out = nc.dram_tensor("out", [128, SUBS * NT, D], bf, kind="ExternalOutput")

    with tile.TileContext(nc) as tc, ExitStack() as ctx:
        singles = ctx.enter_context(tc.tile_pool(name="singles", bufs=1))
        xp_pool = ctx.enter_context(tc.tile_pool(name="xp", bufs=3))
        exp_pool = ctx.enter_context(tc.tile_pool(name="exp", bufs=4))
        sm_pool = ctx.enter_context(tc.tile_pool(name="small", bufs=6))
        a_pool = ctx.enter_context(tc.tile_pool(name="a", bufs=4))
        # vlc tiles stay alive for a norm batch PLUS the next one (scales
        # are deferred and interleaved into the following batch)
        vlc_pool = ctx.enter_context(tc.tile_pool(name="vlc", bufs=68))
        sq_pool = ctx.enter_context(tc.tile_pool(name="sq", bufs=2))
        st_pool = ctx.enter_context(tc.tile_pool(name="st", bufs=2))
        out_pool = ctx.enter_context(tc.tile_pool(name="vout", bufs=2))
        lg_ps = ctx.enter_context(tc.tile_pool(name="lg", bufs=3, space="PSUM"))
        vl_ps = ctx.enter_context(tc.tile_pool(name="vl", bufs=3, space="PSUM"))
        as_ps = ctx.enter_context(tc.tile_pool(name="as", bufs=2, space="PSUM"))

        # params go on the scalar queue so they overlap the first x load
        cp_t = singles.tile([128, 4, KT], bf)
        nc.scalar.dma_start(out=cp_t, in_=cp[:, :, :])
        bias2_t = singles.tile([2, 4 * KT], bf)
        nc.scalar.dma_start(out=bias2_t, in_=bias2[:, :])
        c2_t = singles.tile([128, D], bf)
        nc.scalar.dma_start(out=c2_t, in_=c2t4[:, :])
        ones2_t = singles.tile([2, 128], bf)
        nc.vector.memset(ones2_t, 1.0)
        nones_t = singles.tile([128, 1], bf)
        nc.vector.memset(nones_t, -1.0)
        eps_t = singles.tile([128, 1], f32)
        nc.vector.memset(eps_t, 1e-20)

        # uniform norm batches: one Sqrt table swap pair per 32 subgroups
        sched = [8, 8, 8, 8]
        assert sum(sched) == NT
        t0 = 0
        # deferred scale+store closures from the previous norm batch; emitted
        # one per subgroup so the DVE never sees a 32-op burst that would
        # stall STTs (and through PSUM, the PE) at batch boundaries
        pending = []
        for nb in sched:
            vlcs = []
            ss_t = st_pool.tile([128, SUBS * NB], f32, tag="ss")
            for t in range(t0, t0 + nb):
                xp_t = xp_pool.tile([128, G, D + 512], f8)
                nc.sync.dma_start(out=xp_t, in_=xp[t, :, :, :])

                for h in range(SUBS):  # 4-sample subgroups per group
                    j = SUBS * (t - t0) + h
                    lg = lg_ps.tile([128, 4, KT], f32)
                    # bias: logits += 1*bias_hi + 1*bias_lo (rank-2 matmul)
                    nc.tensor.matmul(
                        lg[:, :, :], ones2_t[:, :], bias2_t[:, :],
                        start=True, stop=False,
                    )
                    for s in range(4):
                        g = 4 * h + s
                        for c in range(4):
                            nc.tensor.matmul(
                                lg[:, s, :],
                                xp_t[:, g, D + 128 * c : D + 128 * (c + 1)],
                                cp_t[:, c, :],
                                start=False, stop=(c == 3),
                            )
                    # softmax over 48 (free dim), no max subtraction needed
                    exp_t = exp_pool.tile([128, 4, KT], f32)
                    nc.scalar.activation(exp_t, lg[:, :, :], AF.Exp)
                    den_t = sm_pool.tile([128, 4], f32)
                    nc.vector.reduce_sum(den_t, exp_t, axis=mybir.AxisListType.X)
                    rec_t = sm_pool.tile([128, 4], f32)
                    nc.vector.reciprocal(rec_t, den_t)
                    # a = exp[:, :, :K] * recip (one op, free-dim broadcast)
                    a_t = a_pool.tile([128, 4, K], bf)
                    rec_ap = rec_t[:, :]
                    rec_bc = bass.AP(
                        rec_ap.tensor, rec_ap.offset,
                        [rec_ap.ap[0], rec_ap.ap[1], [0, K]],
                    )
                    nc.gpsimd.tensor_mul(a_t[:, :, :], exp_t[:, :, 0:K], rec_bc)
                    # vlad = a^T @ x (col-tiled, 4 samples share one PSUM bank)
                    vl = vl_ps.tile([128, D], f32)
                    asm = as_ps.tile([128, 1], f32)
                    for s in range(4):
                        g = 4 * h + s
                        nc.tensor.matmul(
                            vl[32 * s : 32 * s + 32, :],
                            a_t[:, s, :],
                            xp_t[:, g, 0:D],
                            start=True, stop=True,
                            tile_position=(0, 32 * s),
                        )
                    # -asum for all 4 samples in one matmul (lhsT free = 128)
                    nc.tensor.matmul(
                        asm[:, :], a_t[:, :, :], nones_t[:, :],
                        start=True, stop=True,
                    )
                    # vlc = vl + (-asum)*c2t  (asm holds -asum)
                    vlc_t = vlc_pool.tile([128, D], bf)
                    nc.vector.scalar_tensor_tensor(
                        out=vlc_t, in0=c2_t, scalar=asm[:, 0:1], in1=vl[:, :],
                        op0=ALU.mult, op1=ALU.add,
                    )
                    vlcs.append(vlc_t)
                    # ss[:, j] = sum_d vlc^2 (Square is in every ACT table)
                    sq_t = sq_pool.tile([128, D], bf)
                    nc.scalar.activation(
                        sq_t, vlc_t, AF.Square, accum_out=ss_t[:, j : j + 1]
                    )
                    if pending:
                        pending.pop(0)()
            # batched norm: sfac = 1/sqrt(32*ss + tiny); one table swap pair
            nj = SUBS * nb
            den_b = st_pool.tile([128, SUBS * NB], f32, tag="denb")
            nc.scalar.activation(
                den_b[:, 0:nj], ss_t[:, 0:nj], AF.Sqrt, scale=32.0,
                bias=eps_t[:, 0:1],
            )
            sf_b = st_pool.tile([128, SUBS * NB], f32, tag="sfb")
            nc.vector.reciprocal(sf_b[:, 0:nj], den_b[:, 0:nj])
            vn_t = out_pool.tile([128, NB * SUBS, D], bf, tag="vn")

            def mk(j, bt0, bvlcs, bsf, bvn):
                def emit():
                    nc.vector.tensor_scalar_mul(
                        bvn[:, j, :], bvlcs[j], bsf[:, j : j + 1]
                    )
                    if j % 8 == 7 or j == nj - 1:
                        j0 = (j // 8) * 8
                        nc.scalar.dma_start(
                            out=out[:, SUBS * bt0 + j0 : SUBS * bt0 + j + 1, :],
                            in_=bvn[:, j0 : j + 1, :],
                        )
                return emit

            pending = [mk(j, t0, vlcs, sf_b, vn_t) for j in range(nj)]
            t0 += nb
        for fn in pending:
            fn()

    nc.compile()
    return nc


def _get_nc():
    if "nc" not in _NC_CACHE:
        _NC_CACHE["nc"] = _build_nc()
    return _NC_CACHE["nc"]


def _prep_host(x, clusters, bn_w, bn_b, bn_rm, bn_rv, clusters2):
    """Fold BN into weights; build device-layout arrays."""
    s = (1.0 / np.sqrt(bn_rv.astype(np.float64) + BN_EPS)) * bn_w
    s = s.astype(np.float32)
    Cp = (clusters * s[None, :]).astype(np.float32)
    biasp = (bn_b - bn_rm * s).astype(np.float32)

    cp_dev = np.ascontiguousarray(
        (Cp / XSCALE).reshape(4, 128, KT).transpose(1, 0, 2).astype(_F16)
    )
    b_hi = biasp.astype(_F16)
    b_lo = (biasp - b_hi.astype(np.float32)).astype(_F16)
    bias2_dev = np.ascontiguousarray(
        np.stack([np.tile(b_hi, 4), np.tile(b_lo, 4)], axis=0)
    )
    c2t4_dev = np.ascontiguousarray(
        np.tile((clusters2[0].T * XSCALE).astype(_F16), (4, 1))
    )

    xb = (x * XSCALE).astype(_f8())  # [B, N, D]
    xp_list = []
    for c in range(NCORES):
        xc = xb[c * BLOC : (c + 1) * BLOC]
        xpk = np.empty((NT, 128, G, D + 512), dtype=_f8())
        # natural layout block: partition = n
        xpk[:, :, :, 0:D] = xc.reshape(NT, G, 128, D).transpose(0, 2, 1, 3)
        # transposed layout block: partition = d within 128-chunk
        xpk[:, :, :, D:] = (
            xc.reshape(NT, G, 128, 4, 128)
            .transpose(0, 4, 1, 3, 2)
            .reshape(NT, 128, G, 512)
        )
        xp_list.append(xpk)
    return cp_dev, bias2_dev, c2t4_dev, xp_list


def kernel(x, clusters, bn_w, bn_b, bn_rm, bn_rv, clusters2):
    from concourse.bass_utils import run_bass_kernel_spmd

    cp_dev, bias2_dev, c2t4_dev, xp_list = _prep_host(
        x, clusters, bn_w, bn_b, bn_rm, bn_rv, clusters2
    )
    nc = _get_nc()
    in_maps = [
        {
            "xp": xp_list[c],
            "cp": cp_dev,
            "bias2": bias2_dev,
            "c2t4": c2t4_dev,
        }
        for c in range(NCORES)
    ]
    res = run_bass_kernel_spmd(nc, in_maps, core_ids=list(range(NCORES)))
    outs = []
    for c in range(NCORES):
        o = res.results[c]["out"].astype(np.float32)  # [128, SUBS*NT, D]
        o = o.reshape(4, K, SUBS * NT, D).transpose(2, 0, 3, 1).reshape(BLOC, D * K)
        outs.append(o)
    return np.ascontiguousarray(np.concatenate(outs, axis=0))

